# revision 3
# baseline (speedup 1.0000x reference)
"""Trainium2 Bass kernel for the ViT attention block (nn_Attention_17824114278463).

Data-parallel over batch: 64 images -> 8 NeuronCores x 8 images.
Per core (all tokens/channels local):
  patch-embed (fp32r matmuls) -> QKV (q,k transposed layout via bf16 matmuls;
  v natural via fp32r) -> 8-head causal attention (bf16, no max-subtraction --
  logits are O(0.5) for this model scale) -> output projection (fp32r).

Host side does layout prep only: x is repacked to patch-major [pixel, token],
conv_w to [pixel, cout], biases replicated/transposed, causal mask precomputed.
Outputs come back as y [2048,1024] f32 per core, k^T [8,1024,256] bf16 and
v [8,256,1024] bf16 per core; host reassembles (y, present).
"""

import numpy as np
import ml_dtypes

import concourse.bacc as bacc
import concourse.tile as tile
from concourse import mybir
from concourse.bass_utils import run_bass_kernel_spmd

F32 = mybir.dt.float32
F32R = mybir.dt.float32r
BF16 = mybir.dt.bfloat16
AFT = mybir.ActivationFunctionType
ALU = mybir.AluOpType

NCORES = 8
BL = 8            # images per core
T = 256           # tokens per image
C = 1024          # embed dim
NH = 8            # heads
HD = 128          # head dim
NPIX = 256        # pixels per 16x16 patch
SCALE = 1.0 / float(np.sqrt(HD))


def build_nc():
    nc = bacc.Bacc("TRN2", target_bir_lowering=False, debug=False)

    xt = nc.declare_dram_parameter("xt", [NPIX, BL * T], F32R, isOutput=False)
    wc = nc.declare_dram_parameter("wc", [NPIX, C], F32R, isOutput=False)
    wq = nc.declare_dram_parameter("wq", [C, C], BF16, isOutput=False)
    wk = nc.declare_dram_parameter("wk", [C, C], BF16, isOutput=False)
    wv = nc.declare_dram_parameter("wv", [C, C], F32R, isOutput=False)
    wp = nc.declare_dram_parameter("wp", [C, C], F32R, isOutput=False)
    cbt = nc.declare_dram_parameter("cbt", [128, 8], F32, isOutput=False)
    bqt = nc.declare_dram_parameter("bqt", [128, 8], F32, isOutput=False)
    bkt = nc.declare_dram_parameter("bkt", [128, 8], F32, isOutput=False)
    bvr = nc.declare_dram_parameter("bvr", [128, C], F32, isOutput=False)
    bpr = nc.declare_dram_parameter("bpr", [128, C], F32, isOutput=False)
    msk = nc.declare_dram_parameter("msk", [128, 128], BF16, isOutput=False)

    y = nc.declare_dram_parameter("y", [BL * T, C], F32, isOutput=True)
    kt = nc.declare_dram_parameter("kt", [BL, C, T], BF16, isOutput=True)
    vn = nc.declare_dram_parameter("vn", [BL, T, C], BF16, isOutput=True)

    with tile.TileContext(nc) as tc:
        _body(tc, xt, wc, wq, wk, wv, wp, cbt, bqt, bkt, bvr, bpr, msk, y, kt, vn)
    nc.compile()
    return nc


def _body(tc, xt, wc, wq, wk, wv, wp, cbt, bqt, bkt, bvr, bpr, msk, y, kt, vn):
    from contextlib import ExitStack

    nc = tc.nc
    with ExitStack() as ctx:
        const = ctx.enter_context(tc.tile_pool(name="const", bufs=1))
        wpool = ctx.enter_context(tc.tile_pool(name="w", bufs=1))
        work = ctx.enter_context(tc.tile_pool(name="work", bufs=1))
        xtp = ctx.enter_context(tc.tile_pool(name="xtp", bufs=2))
        spool = ctx.enter_context(tc.tile_pool(name="sp", bufs=4))
        rcp = ctx.enter_context(tc.tile_pool(name="rcp", bufs=2))
        yop = ctx.enter_context(tc.tile_pool(name="yop", bufs=3))
        psum = ctx.enter_context(tc.tile_pool(name="ps", bufs=2, space="PSUM"))

        # ---- constants ----
        mk_sb = const.tile([128, 128], BF16, tag="mk")
        nc.gpsimd.dma_start(mk_sb[:], msk.ap())
        ones_bf = const.tile([128, 128], BF16, tag="ones")
        nc.vector.memset(ones_bf[:], 1.0)
        cb_sb = const.tile([128, 8], F32, tag="cb")
        nc.gpsimd.dma_start(cb_sb[:], cbt.ap())
        bq_sb = const.tile([128, 8], F32, tag="bq")
        nc.gpsimd.dma_start(bq_sb[:], bqt.ap())
        bk_sb = const.tile([128, 8], F32, tag="bk")
        nc.gpsimd.dma_start(bk_sb[:], bkt.ap())
        bv_sb = const.tile([128, C], F32, tag="bv")
        nc.gpsimd.dma_start(bv_sb[:], bvr.ap())
        bp_sb = const.tile([128, C], F32, tag="bp")
        nc.gpsimd.dma_start(bp_sb[:], bpr.ap())

        # ---- weights (resident) ----
        wc_sb = wpool.tile([128, 2, C], F32R, tag="wc")
        nc.gpsimd.dma_start(wc_sb[:], wc.ap().rearrange("(k p) c -> p k c", p=128))
        wq_sb = wpool.tile([128, 8, C], BF16, tag="wq")
        nc.gpsimd.dma_start(wq_sb[:], wq.ap().rearrange("(k p) c -> p k c", p=128))
        wk_sb = wpool.tile([128, 8, C], BF16, tag="wk")
        nc.gpsimd.dma_start(wk_sb[:], wk.ap().rearrange("(k p) c -> p k c", p=128))
        wv_sb = wpool.tile([128, 8, C], F32R, tag="wv")
        nc.gpsimd.dma_start(wv_sb[:], wv.ap().rearrange("(k p) c -> p k c", p=128))
        wp_sb = wpool.tile([128, 8, C], F32R, tag="wp")
        nc.gpsimd.dma_start(wp_sb[:], wp.ap().rearrange("(k p) c -> p k c", p=128))

        xt_r = xt.ap().rearrange("(k p) (b t) -> p k b t", p=128, b=BL)

        for b in range(BL):
            # ---- stream in this image's patches: [128, 2, 256] ----
            xt_sb = xtp.tile([128, 2, T], F32R, tag="xt")
            nc.gpsimd.dma_start(xt_sb[:], xt_r[:, :, b, :])

            tokT = work.tile([128, 8, T], F32R, tag="tokT")
            tokB = work.tile([128, 8, T], BF16, tag="tokB")
            qT = work.tile([128, 8, T], BF16, tag="qT")
            kTt = work.tile([128, 8, T], BF16, tag="kTt")
            vnn = work.tile([128, 2, C], BF16, tag="vnn")
            yTt = work.tile([128, 8, T], F32R, tag="yTt")

            # ---- patch embed: tokT[ct] = sum_k wc[k,ct].T @ xt[k] + cb ----
            for ct in range(8):
                ps = psum.tile([128, T], F32, tag="p256")
                for k in range(2):
                    nc.tensor.matmul(
                        ps[:],
                        wc_sb[:, k, ct * 128 : (ct + 1) * 128],
                        xt_sb[:, k, :],
                        start=(k == 0),
                        stop=(k == 1),
                    )
                nc.scalar.activation(
                    tokT[:, ct, :], ps[:], AFT.Identity, bias=cb_sb[:, ct : ct + 1]
                )
                nc.vector.tensor_copy(tokB[:, ct, :], ps[:])

            # ---- q^T, k^T (bf16) ----
            for wsb, bsb, dst in ((wq_sb, bq_sb, qT), (wk_sb, bk_sb, kTt)):
                for ct in range(8):
                    ps = psum.tile([128, T], F32, tag="p256")
                    for k in range(8):
                        nc.tensor.matmul(
                            ps[:],
                            wsb[:, k, ct * 128 : (ct + 1) * 128],
                            tokB[:, k, :],
                            start=(k == 0),
                            stop=(k == 7),
                        )
                    nc.scalar.activation(
                        dst[:, ct, :], ps[:], AFT.Identity, bias=bsb[:, ct : ct + 1]
                    )
            nc.gpsimd.dma_start(
                kt.ap()[b].rearrange("(ct p) t -> p ct t", p=128), kTt[:]
            )

            # ---- v natural (fp32r): v[mt, nt] = sum_k tokT[k,mt].T @ wv[k,nt] ----
            for mt in range(2):
                for nt in range(2):
                    ps = psum.tile([128, 512], F32, tag="p512")
                    for k in range(8):
                        nc.tensor.matmul(
                            ps[:],
                            tokT[:, k, mt * 128 : (mt + 1) * 128],
                            wv_sb[:, k, nt * 512 : (nt + 1) * 512],
                            start=(k == 0),
                            stop=(k == 7),
                        )
                    nc.vector.tensor_tensor(
                        vnn[:, mt, nt * 512 : (nt + 1) * 512],
                        ps[:],
                        bv_sb[:, nt * 512 : (nt + 1) * 512],
                        op=ALU.add,
                    )
            nc.gpsimd.dma_start(
                vn.ap()[b].rearrange("(mt p) c -> p mt c", p=128), vnn[:]
            )

            # ---- attention per head ----
            for h in range(8):
                psS = psum.tile([128, 512], F32, tag="p512")
                nc.tensor.matmul(
                    psS[:, 0:T],
                    kTt[:, h, 0:128],
                    qT[:, h, :],
                    start=True,
                    stop=True,
                )
                nc.tensor.matmul(
                    psS[:, T : T + 128],
                    kTt[:, h, 128:256],
                    qT[:, h, 128:256],
                    start=True,
                    stop=True,
                )
                e0 = spool.tile([128, T], BF16, tag="e0")
                e1 = spool.tile([128, 128], BF16, tag="e1")
                nc.scalar.activation(e0[:], psS[:, 0:T], AFT.Exp, scale=SCALE)
                nc.scalar.activation(
                    e1[:], psS[:, T : T + 128], AFT.Exp, scale=SCALE
                )
                nc.vector.tensor_tensor(
                    e0[:, 0:128], e0[:, 0:128], mk_sb[:], op=ALU.mult
                )
                nc.vector.tensor_tensor(e1[:], e1[:], mk_sb[:], op=ALU.mult)
                pc = psum.tile([128, T], F32, tag="pc")
                nc.tensor.matmul(pc[:], ones_bf[:], e0[:], start=True, stop=False)
                nc.tensor.matmul(
                    pc[:, 128:256], ones_bf[:], e1[:], start=False, stop=True
                )
                py = psum.tile([128, T], F32, tag="py")
                nc.tensor.matmul(
                    py[:], vnn[:, 0, h * 128 : (h + 1) * 128], e0[:],
                    start=True, stop=False,
                )
                nc.tensor.matmul(
                    py[:, 128:256], vnn[:, 1, h * 128 : (h + 1) * 128], e1[:],
                    start=False, stop=True,
                )
                rc = rcp.tile([128, T], F32, tag="rc")
                nc.vector.reciprocal(rc[:], pc[:])
                nc.vector.tensor_tensor(yTt[:, h, :], py[:], rc[:], op=ALU.mult)

            # ---- projection (fp32r) + bias + store ----
            for mt in range(2):
                for nt in range(2):
                    ps = psum.tile([128, 512], F32, tag="p512")
                    for h in range(8):
                        nc.tensor.matmul(
                            ps[:],
                            yTt[:, h, mt * 128 : (mt + 1) * 128],
                            wp_sb[:, h, nt * 512 : (nt + 1) * 512],
                            start=(h == 0),
                            stop=(h == 7),
                        )
                    yo = yop.tile([128, 512], F32, tag="yo")
                    nc.vector.tensor_tensor(
                        yo[:], ps[:], bp_sb[:, nt * 512 : (nt + 1) * 512], op=ALU.add
                    )
                    nc.gpsimd.dma_start(
                        y.ap()[
                            b * T + mt * 128 : b * T + (mt + 1) * 128,
                            nt * 512 : (nt + 1) * 512,
                        ],
                        yo[:],
                    )


def prep_in_maps(x, conv_w, conv_b, Wq, bq, Wk, bk, Wv, bv, Wp, bp):
    """Host-side sharding + layout prep. Returns list of 8 per-core in_maps."""
    B = x.shape[0]
    assert B == NCORES * BL
    # patches^T: [pixel=(ph,pw), (b, i, j)]
    xr = np.ascontiguousarray(
        x.reshape(B, 16, 16, 16, 16).transpose(2, 4, 0, 1, 3).reshape(NPIX, B * T)
    ).astype(np.float32)
    wc_h = np.ascontiguousarray(conv_w.reshape(C, NPIX).T).astype(np.float32)
    bf = ml_dtypes.bfloat16
    wq_h = Wq.astype(bf)
    wk_h = Wk.astype(bf)
    wv_h = np.ascontiguousarray(Wv).astype(np.float32)
    wp_h = np.ascontiguousarray(Wp).astype(np.float32)
    cbt_h = np.ascontiguousarray(conv_b.reshape(8, 128).T).astype(np.float32)
    bqt_h = np.ascontiguousarray(bq.reshape(8, 128).T).astype(np.float32)
    bkt_h = np.ascontiguousarray(bk.reshape(8, 128).T).astype(np.float32)
    bvr_h = np.ascontiguousarray(np.broadcast_to(bv, (128, C))).astype(np.float32)
    bpr_h = np.ascontiguousarray(np.broadcast_to(bp, (128, C))).astype(np.float32)
    msk_h = np.triu(np.ones((128, 128), np.float32)).astype(bf)

    in_maps = []
    for c in range(NCORES):
        in_maps.append(
            {
                "xt": np.ascontiguousarray(xr[:, c * BL * T : (c + 1) * BL * T]),
                "wc": wc_h,
                "wq": wq_h,
                "wk": wk_h,
                "wv": wv_h,
                "wp": wp_h,
                "cbt": cbt_h,
                "bqt": bqt_h,
                "bkt": bkt_h,
                "bvr": bvr_h,
                "bpr": bpr_h,
                "msk": msk_h,
            }
        )
    return in_maps


def assemble(results):
    """Gather per-core outputs into (y, present)."""
    y_full = np.concatenate(
        [np.asarray(r["y"], np.float32).reshape(BL, T, C) for r in results], axis=0
    )
    k_full = np.concatenate([np.asarray(r["kt"]) for r in results], axis=0)
    k_full = (
        k_full.reshape(NCORES * BL, NH, HD, T).transpose(0, 1, 3, 2).astype(np.float32)
    )
    v_full = np.concatenate([np.asarray(r["vn"]) for r in results], axis=0)
    v_full = (
        v_full.reshape(NCORES * BL, T, NH, HD).transpose(0, 2, 1, 3).astype(np.float32)
    )
    present = np.stack([k_full, v_full])
    return y_full, present


_NC_CACHE = []


def _get_nc():
    if not _NC_CACHE:
        _NC_CACHE.append(build_nc())
    return _NC_CACHE[0]


def kernel(x, conv_w, conv_b, Wq, bq, Wk, bk, Wv, bv, Wp, bp):
    args = [
        np.asarray(a)
        for a in (x, conv_w, conv_b, Wq, bq, Wk, bk, Wv, bv, Wp, bp)
    ]
    in_maps = prep_in_maps(*args)
    nc = _get_nc()
    res = run_bass_kernel_spmd(nc, in_maps, core_ids=list(range(NCORES)))
    return assemble(res.results)


# revision 11
# speedup vs baseline: 144.4771x; 144.4771x over previous
"""Trainium2 Bass kernel for the ViT attention block (nn_Attention_17824114278463).

Data-parallel over batch: 64 images -> 8 NeuronCores x 8 images each.
Per core, per image:
  patch-embed (bf16) -> V natural (bf16) -> per-head Q^T/K^T (bf16) software-
  pipelined with causal attention (bf16, no max-subtraction -- logits are
  O(0.5) at this model scale) -> output projection (fp32r).

The emission order software-pipelines phases so the TensorEngine never waits
on the ACT/DVE softmax chain: S(h-1) and colsum/AV(h-2) are interleaved into
the Q/K loop, and the next image's patch embed covers the attention tail.

Host side does layout/sharding prep only: x repacked to patch-major
[pixel, token], conv_w to [pixel, cout], biases transposed/replicated,
causal mask precomputed. Outputs per core: y [2048,1024] f32,
k^T [8,1024,256] bf16, v [8,256,1024] bf16; host reassembles (y, present).
"""

import numpy as np
import ml_dtypes

import concourse.bacc as bacc
import concourse.tile as tile
from concourse import mybir
from concourse.bass_utils import run_bass_kernel_spmd

F32 = mybir.dt.float32
F32R = mybir.dt.float32r
BF16 = mybir.dt.bfloat16
AFT = mybir.ActivationFunctionType
ALU = mybir.AluOpType

NCORES = 8
BL = 8            # images per core
T = 256           # tokens per image
C = 1024          # embed dim
NH = 8            # heads
HD = 128          # head dim
NPIX = 256        # pixels per 16x16 patch
SCALE = 1.0 / float(np.sqrt(HD))


def build_nc(repeat=1):
    nc = bacc.Bacc("TRN2", target_bir_lowering=False, debug=False)

    xt = nc.declare_dram_parameter("xt", [NPIX, BL * T], BF16, isOutput=False)
    wc = nc.declare_dram_parameter("wc", [NPIX, C], BF16, isOutput=False)
    wq = nc.declare_dram_parameter("wq", [C, C], BF16, isOutput=False)
    wk = nc.declare_dram_parameter("wk", [C, C], BF16, isOutput=False)
    wv = nc.declare_dram_parameter("wv", [C, C], BF16, isOutput=False)
    wp = nc.declare_dram_parameter("wp", [C, C], F32R, isOutput=False)
    b3 = nc.declare_dram_parameter("b3", [128, 24], F32, isOutput=False)
    bvr = nc.declare_dram_parameter("bvr", [128, C], F32, isOutput=False)
    bpr = nc.declare_dram_parameter("bpr", [128, C], F32, isOutput=False)
    msk = nc.declare_dram_parameter("msk", [128, 128], BF16, isOutput=False)

    y = nc.declare_dram_parameter("y", [BL * T, C], F32, isOutput=True)
    kt = nc.declare_dram_parameter("kt", [BL, C, T], BF16, isOutput=True)
    vn = nc.declare_dram_parameter("vn", [BL, T, C], BF16, isOutput=True)

    with tile.TileContext(nc) as tc:
        _body(tc, xt, wc, wq, wk, wv, wp, b3, bvr, bpr, msk, y, kt, vn, repeat)
    nc.compile()
    return nc


def _body(tc, xt, wc, wq, wk, wv, wp, b3, bvr, bpr, msk, y, kt, vn, repeat=1):
    from contextlib import ExitStack

    nc = tc.nc
    with ExitStack() as ctx:
        const = ctx.enter_context(tc.tile_pool(name="const", bufs=1))
        wpool = ctx.enter_context(tc.tile_pool(name="w", bufs=1))
        work = ctx.enter_context(tc.tile_pool(name="work", bufs=2))
        xtp = ctx.enter_context(tc.tile_pool(name="xtp", bufs=3))
        spool = ctx.enter_context(tc.tile_pool(name="sp", bufs=4))
        rcp = ctx.enter_context(tc.tile_pool(name="rcp", bufs=2))
        yop = ctx.enter_context(tc.tile_pool(name="yop", bufs=3))
        psum = ctx.enter_context(tc.tile_pool(name="ps", bufs=2, space="PSUM"))

        # ---- small constants (one DMA each, needed within ~10us) ----
        b3_sb = const.tile([128, 24], F32, tag="b3")
        nc.gpsimd.dma_start(b3_sb[:], b3.ap())
        cb_sb = b3_sb[:, 0:8]
        bq_sb = b3_sb[:, 8:16]
        bk_sb = b3_sb[:, 16:24]
        mk_sb = const.tile([128, 128], BF16, tag="mk")
        nc.gpsimd.dma_start(mk_sb[:], msk.ap())
        ones_bf = const.tile([128, 128], BF16, tag="ones")
        nc.vector.memset(ones_bf[:], 1.0)

        xt_r = xt.ap().rearrange("(k p) (b t) -> p k b t", p=128, b=BL)
        blist = [b for _ in range(repeat) for b in range(BL)]
        n_img = len(blist)

        def load_xt(b):
            t = xtp.tile([128, 2, T], BF16, tag="xt")
            nc.gpsimd.dma_start(t[:], xt_r[:, :, b, :])
            return t

        xt_tiles = {0: load_xt(blist[0]), 1: load_xt(blist[1 % n_img])}

        # ---- weights (resident), ordered by first use ----
        wc_sb = wpool.tile([128, 2, C], BF16, tag="wc")
        nc.gpsimd.dma_start(wc_sb[:], wc.ap().rearrange("(k p) c -> p k c", p=128))
        wv_sb = wpool.tile([128, 8, C], BF16, tag="wv")
        nc.gpsimd.dma_start(wv_sb[:], wv.ap().rearrange("(k p) c -> p k c", p=128))
        wq_sb = wpool.tile([128, 8, C], BF16, tag="wq")
        nc.gpsimd.dma_start(wq_sb[:], wq.ap().rearrange("(k p) c -> p k c", p=128))
        wk_sb = wpool.tile([128, 8, C], BF16, tag="wk")
        nc.gpsimd.dma_start(wk_sb[:], wk.ap().rearrange("(k p) c -> p k c", p=128))
        bv_sb = const.tile([128, C], F32, tag="bv")
        nc.gpsimd.dma_start(bv_sb[:], bvr.ap())
        wp_sb = wpool.tile([128, 8, C], F32R, tag="wp")
        nc.gpsimd.dma_start(wp_sb[:], wp.ap().rearrange("(k p) c -> p k c", p=128))
        bp_sb = const.tile([128, C], F32, tag="bp")
        nc.gpsimd.dma_start(bp_sb[:], bpr.ap())

        # per-image state carried across the pipelined emission
        state = {}

        def emit_patch(i):
            """Patch embed for image index i -> tokB tile."""
            xt_sb = xt_tiles.pop(i)
            tokB = work.tile([128, 8, T], BF16, tag="tokB")
            for ct in range(8):
                ps = psum.tile([128, T], F32, tag="p256")
                for k in range(2):
                    nc.tensor.matmul(
                        ps[:],
                        wc_sb[:, k, ct * 128 : (ct + 1) * 128],
                        xt_sb[:, k, :],
                        start=(k == 0),
                        stop=(k == 1),
                    )
                nc.scalar.activation(
                    tokB[:, ct, :], ps[:], AFT.Identity, bias=cb_sb[:, ct : ct + 1]
                )
            state[i] = tokB

        emit_patch(0)

        for i, b in enumerate(blist):
            tokB = state.pop(i)
            if i + 2 < n_img:
                xt_tiles[i + 2] = load_xt(blist[i + 2])

            qT = work.tile([128, 8, T], BF16, tag="qT")
            kTt = work.tile([128, 8, T], BF16, tag="kTt")
            vnn = work.tile([128, 2, C], BF16, tag="vnn")
            yTt = work.tile([128, 8, T], F32R, tag="yTt")

            # ---- v natural (bf16) ----
            for mt in range(2):
                for nt in range(2):
                    ps = psum.tile([128, 512], F32, tag="p512")
                    for k in range(8):
                        nc.tensor.matmul(
                            ps[:],
                            tokB[:, k, mt * 128 : (mt + 1) * 128],
                            wv_sb[:, k, nt * 512 : (nt + 1) * 512],
                            start=(k == 0),
                            stop=(k == 7),
                        )
                    nc.vector.tensor_tensor(
                        vnn[:, mt, nt * 512 : (nt + 1) * 512],
                        ps[:],
                        bv_sb[:, nt * 512 : (nt + 1) * 512],
                        op=ALU.add,
                    )
            nc.gpsimd.dma_start(
                vn.ap()[b].rearrange("(mt p) c -> p mt c", p=128), vnn[:]
            )

            # ---- attention sub-emitters (software-pipelined into qk loop) ----
            psS = {}
            e0s = {}
            e1s = {}
            pcs = {}
            pys = {}

            def emit_S(h):
                ps = psum.tile([128, 512], F32, tag="p512")
                nc.tensor.matmul(
                    ps[:, 0:T], kTt[:, h, 0:128], qT[:, h, :], start=True, stop=True
                )
                nc.tensor.matmul(
                    ps[:, T : T + 128],
                    kTt[:, h, 128:256],
                    qT[:, h, 128:256],
                    start=True,
                    stop=True,
                )
                e0 = spool.tile([128, T], BF16, tag="e0")
                e1 = spool.tile([128, 128], BF16, tag="e1")
                nc.scalar.activation(e0[:], ps[:, 0:T], AFT.Exp, scale=SCALE)
                nc.scalar.activation(e1[:], ps[:, T : T + 128], AFT.Exp, scale=SCALE)
                nc.vector.tensor_tensor(
                    e0[:, 0:128], e0[:, 0:128], mk_sb[:], op=ALU.mult
                )
                nc.vector.tensor_tensor(e1[:], e1[:], mk_sb[:], op=ALU.mult)
                psS[h] = ps
                e0s[h] = e0
                e1s[h] = e1

            def emit_pcpy(h):
                e0, e1 = e0s.pop(h), e1s.pop(h)
                pc = psum.tile([128, T], F32, tag="pc")
                nc.tensor.matmul(pc[:], ones_bf[:], e0[:], start=True, stop=False)
                nc.tensor.matmul(
                    pc[:, 128:256], ones_bf[:], e1[:], start=False, stop=True
                )
                py = psum.tile([128, T], F32, tag="py")
                nc.tensor.matmul(
                    py[:], vnn[:, 0, h * 128 : (h + 1) * 128], e0[:],
                    start=True, stop=False,
                )
                nc.tensor.matmul(
                    py[:, 128:256], vnn[:, 1, h * 128 : (h + 1) * 128], e1[:],
                    start=False, stop=True,
                )
                rc = rcp.tile([128, T], F32, tag="rc")
                nc.vector.reciprocal(rc[:], pc[:])
                nc.vector.tensor_tensor(yTt[:, h, :], py[:], rc[:], op=ALU.mult)
                psS.pop(h)

            # ---- q^T/k^T per head, with S(h-1), pcpy(h-2) interleaved ----
            for h in range(8):
                for wsb, bsb, dst in ((wq_sb, bq_sb, qT), (wk_sb, bk_sb, kTt)):
                    ps = psum.tile([128, T], F32, tag="p256")
                    for k in range(8):
                        nc.tensor.matmul(
                            ps[:],
                            wsb[:, k, h * 128 : (h + 1) * 128],
                            tokB[:, k, :],
                            start=(k == 0),
                            stop=(k == 7),
                        )
                    nc.scalar.activation(
                        dst[:, h, :], ps[:], AFT.Identity, bias=bsb[:, h : h + 1]
                    )
                if h >= 1:
                    emit_S(h - 1)
                if h >= 2:
                    emit_pcpy(h - 2)
            nc.gpsimd.dma_start(
                kt.ap()[b].rearrange("(ct p) t -> p ct t", p=128), kTt[:]
            )
            emit_S(7)
            emit_pcpy(6)
            if i + 1 < n_img:
                emit_patch(i + 1)
            emit_pcpy(7)

            # ---- projection (fp32r) + bias + store ----
            for mt in range(2):
                for nt in range(2):
                    ps = psum.tile([128, 512], F32, tag="p512")
                    for h in range(8):
                        nc.tensor.matmul(
                            ps[:],
                            yTt[:, h, mt * 128 : (mt + 1) * 128],
                            wp_sb[:, h, nt * 512 : (nt + 1) * 512],
                            start=(h == 0),
                            stop=(h == 7),
                        )
                    yo = yop.tile([128, 512], F32, tag="yo")
                    nc.vector.tensor_tensor(
                        yo[:], ps[:], bp_sb[:, nt * 512 : (nt + 1) * 512], op=ALU.add
                    )
                    nc.gpsimd.dma_start(
                        y.ap()[
                            b * T + mt * 128 : b * T + (mt + 1) * 128,
                            nt * 512 : (nt + 1) * 512,
                        ],
                        yo[:],
                    )


def prep_in_maps(x, conv_w, conv_b, Wq, bq, Wk, bk, Wv, bv, Wp, bp):
    """Host-side sharding + layout prep. Returns list of 8 per-core in_maps."""
    B = x.shape[0]
    assert B == NCORES * BL
    bf = ml_dtypes.bfloat16
    # patches^T: [pixel=(ph,pw), (b, i, j)]
    xr = np.ascontiguousarray(
        x.reshape(B, 16, 16, 16, 16).transpose(2, 4, 0, 1, 3).reshape(NPIX, B * T)
    ).astype(bf)
    wc_h = np.ascontiguousarray(conv_w.reshape(C, NPIX).T).astype(bf)
    wq_h = Wq.astype(bf)
    wk_h = Wk.astype(bf)
    wv_h = Wv.astype(bf)
    wp_h = np.ascontiguousarray(Wp).astype(np.float32)
    b3_h = np.stack(
        [conv_b.reshape(8, 128), bq.reshape(8, 128), bk.reshape(8, 128)]
    ).reshape(24, 128).T.astype(np.float32)
    b3_h = np.ascontiguousarray(b3_h)
    bvr_h = np.ascontiguousarray(np.broadcast_to(bv, (128, C))).astype(np.float32)
    bpr_h = np.ascontiguousarray(np.broadcast_to(bp, (128, C))).astype(np.float32)
    msk_h = np.triu(np.ones((128, 128), np.float32)).astype(bf)

    in_maps = []
    for c in range(NCORES):
        in_maps.append(
            {
                "xt": np.ascontiguousarray(xr[:, c * BL * T : (c + 1) * BL * T]),
                "wc": wc_h,
                "wq": wq_h,
                "wk": wk_h,
                "wv": wv_h,
                "wp": wp_h,
                "b3": b3_h,
                "bvr": bvr_h,
                "bpr": bpr_h,
                "msk": msk_h,
            }
        )
    return in_maps


def assemble(results):
    """Gather per-core outputs into (y, present)."""
    y_full = np.concatenate(
        [np.asarray(r["y"], np.float32).reshape(BL, T, C) for r in results], axis=0
    )
    k_full = np.concatenate([np.asarray(r["kt"]) for r in results], axis=0)
    k_full = (
        k_full.reshape(NCORES * BL, NH, HD, T).transpose(0, 1, 3, 2).astype(np.float32)
    )
    v_full = np.concatenate([np.asarray(r["vn"]) for r in results], axis=0)
    v_full = (
        v_full.reshape(NCORES * BL, T, NH, HD).transpose(0, 2, 1, 3).astype(np.float32)
    )
    present = np.stack([k_full, v_full])
    return y_full, present


_NC_CACHE = []


def _get_nc():
    if not _NC_CACHE:
        _NC_CACHE.append(build_nc())
    return _NC_CACHE[0]


def kernel(x, conv_w, conv_b, Wq, bq, Wk, bk, Wv, bv, Wp, bp):
    args = [
        np.asarray(a)
        for a in (x, conv_w, conv_b, Wq, bq, Wk, bk, Wv, bv, Wp, bp)
    ]
    in_maps = prep_in_maps(*args)
    nc = _get_nc()
    res = run_bass_kernel_spmd(nc, in_maps, core_ids=list(range(NCORES)))
    return assemble(res.results)
